# revision 1
# baseline (speedup 1.0000x reference)
"""Trainium2 Bass kernel for nn_Net1_45930380263895 (retrieval_knn).

Math (reference): for each batch b and codebook entry c,
  D[t,j]   = ||traj[b,t] - Y[c,:,j]||^2          (only t<256 is ever used:
                                                  gather indices come from
                                                  Aset which is int in [0,256))
  phi      = exp(-D/256) gathered at (t,j) index pairs from Aset -> (nA, L)
  odds     = sum_a mean_l( phi * softmax_l(-phi/0.1) )
  out      = selu(odds @ W.T + b)

Device strategy (8 NeuronCores, core s = (b = s//2, c-half = s%2)):
  - One matmul per block of 8 consecutive j's: stationary operand columns are
    arranged as m = (j&7)*16 + (c%16) so the PSUM/table partition layout is
    (j-low-3-bits, c). Contraction is augmented to K=66 so the matmul directly
    produces D = x2[t] + y2[c,j] - 2*xy (rows: trajT, x2, ones).
  - The D-table lives in SBUF as (128 partitions, 1 + 32*256) f32 with flat
    free index 1 + (j>>3)*256 + t. Free slot 0 holds -2560 so padded gather
    slots produce phi = e^10 whose softmin terms underflow to exactly 0.
  - GPSIMD ap_gather: gpsimd core k (partitions 16k..16k+15) gathers the
    host-bucketed index list for j&7 == k (per alignment set a, padded to 256
    slots), so every gathered lane is a needed (c, t, j) triple.
  - ACT computes both exps, DVE multiplies and segment-reduces to per-(a)
    num/den partials; the tiny (128,8) result is combined on the host with
    the final linear + SELU.
"""

import numpy as np

B, T_USED, D_DIM, Q = 4, 256, 64, 256
CTRAJ, NA, L, C_OUT = 32, 4, 1024, 2
K_AUG = 66                      # 64 d + x2 row + ones/y2 row
NJB = 32                        # j-blocks of 8
P_SEG = 256                     # padded slots per (a, j-group) bucket
NIDX = NA * P_SEG               # gather indices per gpsimd core
NE = 1 + NJB * 256              # table elems per partition (zero slot + data)
MASK_D = -2560.0                # masked D -> phi=e^10 -> softmin terms == 0
SELU_L = 1.0507009873554805
SELU_A = 1.6732632423543772

_CACHE = {}


def _build_nc():
    import concourse.bacc as bacc
    import concourse.tile as tile
    from concourse import mybir

    f32 = mybir.dt.float32
    nc = bacc.Bacc(None, target_bir_lowering=False)
    rhs_d = nc.dram_tensor("rhs_aug", (K_AUG, 256), f32, kind="ExternalInput")
    w_d = nc.dram_tensor("lhsT_all", (K_AUG, NJB * 128), f32, kind="ExternalInput")
    idx_d = nc.dram_tensor("idxs", (128, NIDX // 16), mybir.dt.int16,
                           kind="ExternalInput")
    res_d = nc.dram_tensor("res", (128, NA * 2), f32, kind="ExternalOutput")

    with tile.TileContext(nc) as tc:
        with tc.tile_pool(name="const", bufs=1) as const, \
             tc.tile_pool(name="wp", bufs=2) as wp, \
             tc.tile_pool(name="psp", bufs=4, space="PSUM") as psp, \
             tc.tile_pool(name="post", bufs=1) as post:
            rhs_sb = const.tile([K_AUG, 256], f32)
            idx_sb = const.tile([128, NIDX // 16], mybir.dt.int16)
            table = const.tile([128, NE], f32)
            nc.sync.dma_start(out=rhs_sb[:, :], in_=rhs_d[:, :])
            nc.sync.dma_start(out=idx_sb[:, :], in_=idx_d[:, :])
            nc.vector.memset(table[:, 0:1], MASK_D)

            for chunk in range(4):
                w_sb = wp.tile([K_AUG, 8 * 128], f32, tag="w")
                nc.sync.dma_start(
                    out=w_sb[:, :], in_=w_d[:, chunk * 1024:(chunk + 1) * 1024])
                for i in range(8):
                    jb = chunk * 8 + i
                    pst = psp.tile([128, 256], f32)
                    nc.tensor.matmul(pst[:, :], lhsT=w_sb[:, i * 128:(i + 1) * 128],
                                     rhs=rhs_sb[:, :], start=True, stop=True)
                    dst = table[:, 1 + jb * 256:1 + (jb + 1) * 256]
                    if jb % 2 == 0:
                        nc.vector.tensor_copy(dst, pst[:, :])
                    else:
                        nc.scalar.copy(dst, pst[:, :])

            g = post.tile([128, NIDX], f32)
            nc.gpsimd.ap_gather(out_ap=g[:, :], in_ap=table[:, :],
                                idxs_ap=idx_sb[:, :], channels=128,
                                num_elems=NE, d=1, num_idxs=NIDX)
            phi = post.tile([128, NIDX], f32)
            nc.scalar.activation(phi[:, :], g[:, :],
                                 mybir.ActivationFunctionType.Exp,
                                 scale=-1.0 / 256.0)
            ephi = post.tile([128, NIDX], f32)
            nc.scalar.activation(ephi[:, :], phi[:, :],
                                 mybir.ActivationFunctionType.Exp, scale=-10.0)
            prod = post.tile([128, NIDX], f32)
            nc.vector.tensor_mul(prod[:, :], phi[:, :], ephi[:, :])
            res_sb = post.tile([128, NA, 2], f32)
            prod_v = prod[:, :].rearrange("p (a l) -> p a l", a=NA)
            ephi_v = ephi[:, :].rearrange("p (a l) -> p a l", a=NA)
            nc.vector.tensor_reduce(res_sb[:, :, 0], prod_v,
                                    axis=mybir.AxisListType.X,
                                    op=mybir.AluOpType.add)
            nc.vector.tensor_reduce(res_sb[:, :, 1], ephi_v,
                                    axis=mybir.AxisListType.X,
                                    op=mybir.AluOpType.add)
            nc.sync.dma_start(out=res_d[:, :],
                              in_=res_sb[:, :].rearrange("p a k -> p (a k)"))
    nc.compile()
    return nc


def _prep_core_inputs(traj, Aset, Y):
    """Host-side shard prep: augmented operands + bucketed gather indices."""
    rhs_all, w_all, idx_all = [], [], []
    for bb in range(B):
        tr = traj[bb, :T_USED, :].astype(np.float32)
        x2 = (tr * tr).sum(-1)
        rhs = np.concatenate(
            [tr.T, x2[None], np.ones((1, T_USED), np.float32)], 0)
        rhs_all.append(np.ascontiguousarray(rhs, np.float32))

        t_l = Aset[bb, :, :, 0].astype(np.int64)
        j_l = Aset[bb, :, :, 1].astype(np.int64)
        idx_np = np.zeros((128, NIDX // 16), np.int16)
        for jg in range(8):
            lst = np.zeros(NIDX, np.int16)
            for a in range(NA):
                sel = np.nonzero((j_l[a] & 7) == jg)[0]
                if len(sel) > P_SEG:
                    return None, None, None  # bucket overflow -> fallback
                v = 1 + (j_l[a, sel] >> 3) * 256 + t_l[a, sel]
                lst[a * P_SEG:a * P_SEG + len(sel)] = v.astype(np.int16)
            idx_np[16 * jg:16 * jg + 16, :] = lst.reshape(NIDX // 16, 16).T
        idx_all.append(idx_np)

    for s in range(8):
        bb, ch = s // 2, s % 2
        Yh = Y[ch * 16:(ch + 1) * 16].astype(np.float32)    # (16, 64, 256)
        y2 = (Yh * Yh).sum(1)                               # (16, 256)
        lhsT = np.zeros((K_AUG, NJB, 8, 16), np.float32)    # k, jb, jg, cc
        lhsT[:64] = -2.0 * Yh.reshape(16, 64, NJB, 8).transpose(1, 2, 3, 0)
        lhsT[64] = 1.0
        lhsT[65] = y2.reshape(16, NJB, 8).transpose(1, 2, 0)
        w_all.append(np.ascontiguousarray(lhsT.reshape(K_AUG, NJB * 128)))
    return rhs_all, w_all, idx_all


def _kernel_numpy_fallback(traj, Aset, Y, W, b):
    q = Y.shape[2]
    flat = Aset[..., 0].astype(np.int64) * q + Aset[..., 1]
    out = np.zeros((traj.shape[0], W.shape[0]), np.float32)
    for bb in range(traj.shape[0]):
        x2 = (traj[bb] * traj[bb]).sum(-1)
        odds = np.zeros(Y.shape[0], np.float32)
        for c in range(Y.shape[0]):
            y2 = (Y[c] * Y[c]).sum(0)
            xy = traj[bb] @ Y[c]
            D = x2[:, None] + y2[None, :] - 2.0 * xy
            phi = np.exp(-D / 256.0).reshape(-1)[flat[bb]]
            e = np.exp(-phi / 0.1)
            odds[c] = (phi * e / e.sum(-1, keepdims=True)).mean(-1).sum()
        z = odds @ W.T + b
        out[bb] = np.where(z > 0, SELU_L * z, SELU_L * SELU_A * np.expm1(z))
    return out.astype(np.float32)


def kernel(traj, Aset, Y, W, b):
    traj = np.asarray(traj, np.float32)
    Aset = np.asarray(Aset)
    Y = np.asarray(Y, np.float32)
    W = np.asarray(W, np.float32)
    b = np.asarray(b, np.float32)

    if (traj.shape != (B, 4096, D_DIM) or Aset.shape != (B, NA, L, 2)
            or Y.shape != (CTRAJ, D_DIM, Q) or Aset.min() < 0
            or Aset.max() >= 256):
        return _kernel_numpy_fallback(traj, Aset, Y, W, b)

    rhs_all, w_all, idx_all = _prep_core_inputs(traj, Aset, Y)
    if rhs_all is None:
        return _kernel_numpy_fallback(traj, Aset, Y, W, b)

    if "nc" not in _CACHE:
        _CACHE["nc"] = _build_nc()
    nc = _CACHE["nc"]

    from concourse import bass_utils
    in_maps = []
    for s in range(8):
        bb, ch = s // 2, s % 2
        in_maps.append({"rhs_aug": rhs_all[bb], "lhsT_all": w_all[s],
                        "idxs": idx_all[bb]})
    res = bass_utils.run_bass_kernel_spmd(nc, in_maps, core_ids=list(range(8)))

    num = np.zeros((B, NA, CTRAJ), np.float64)
    den = np.zeros((B, NA, CTRAJ), np.float64)
    for s in range(8):
        bb, ch = s // 2, s % 2
        r = res.results[s]["res"].reshape(8, 16, NA, 2)
        num[bb, :, ch * 16:(ch + 1) * 16] += r[..., 0].sum(0).T
        den[bb, :, ch * 16:(ch + 1) * 16] += r[..., 1].sum(0).T

    odds = (num / den / L).sum(1)                     # (B, CTRAJ)
    z = (odds @ W.T.astype(np.float64) + b).astype(np.float32)
    return np.where(z > 0, SELU_L * z,
                    SELU_L * SELU_A * np.expm1(z)).astype(np.float32)


# revision 2
# speedup vs baseline: 1.2704x; 1.2704x over previous
"""Trainium2 Bass kernel for nn_Net1_45930380263895 (retrieval_knn).

Math (reference): for each batch b and codebook entry c,
  D[t,j]   = ||traj[b,t] - Y[c,:,j]||^2          (only t<256 is ever used:
                                                  gather indices come from
                                                  Aset which is int in [0,256))
  phi      = exp(-D/256) gathered at (t,j) index pairs from Aset -> (nA, L)
  odds     = sum_a mean_l( phi * softmax_l(-phi/0.1) )
  out      = selu(odds @ W.T + b)

Device strategy (8 NeuronCores, core s = (b = s//2, c-half = s%2)):
  - One bf16 matmul per block of 8 consecutive j's: stationary operand columns
    are arranged as m = (j&7)*16 + (c%16) so the PSUM/table partition layout
    is (j-low-3-bits, c). Contraction is augmented to K=68 so the matmul
    directly produces D = x2[t] + y2[c,j] - 2*xy; the large-magnitude x2/y2
    terms ride as bf16 hi+lo pairs to keep f32-level accuracy.
  - The D-table lives in SBUF as (128 partitions, 1 + 32*256) f32 with flat
    free index 1 + (j>>3)*256 + t. Free slot 0 holds -2560 so padded gather
    slots produce phi = e^10 whose softmin terms underflow to exactly 0.
  - GPSIMD ap_gather: gpsimd core k (partitions 16k..16k+15) gathers the
    host-bucketed index list for j&7 == k (per alignment set a, padded to
    P_SEG slots), so every gathered lane is a needed (c, t, j) triple.
  - ACT computes both exps, DVE multiplies and segment-reduces to per-(a)
    num/den partials; the tiny (128,8) result is combined on the host with
    the final linear + SELU.
"""

import numpy as np

B, T_USED, D_DIM, Q = 4, 256, 64, 256
CTRAJ, NA, L, C_OUT = 32, 4, 1024, 2
K_AUG = 68                      # 64 d + x2_hi/x2_lo + y2_hi/y2_lo rows
NJB = 32                        # j-blocks of 8
P_SEG = 192                     # padded slots per (a, j-group) bucket
NIDX = NA * P_SEG               # gather indices per gpsimd core
NE = 1 + NJB * 256              # table elems per partition (zero slot + data)
MASK_D = -2560.0                # masked D -> phi=e^10 -> softmin terms == 0
SELU_L = 1.0507009873554805
SELU_A = 1.6732632423543772

_CACHE = {}


def _build_nc():
    import concourse.bacc as bacc
    import concourse.tile as tile
    from concourse import mybir

    f32 = mybir.dt.float32
    bf16 = mybir.dt.bfloat16
    nc = bacc.Bacc(None, target_bir_lowering=False)
    rhs_d = nc.dram_tensor("rhs_aug", (K_AUG, 256), bf16, kind="ExternalInput")
    w_d = nc.dram_tensor("lhsT_all", (K_AUG, NJB * 128), bf16,
                         kind="ExternalInput")
    idx_d = nc.dram_tensor("idxs", (128, NIDX // 16), mybir.dt.int16,
                           kind="ExternalInput")
    res_d = nc.dram_tensor("res", (128, NA * 2), f32, kind="ExternalOutput")

    with tile.TileContext(nc) as tc:
        with tc.tile_pool(name="const", bufs=1) as const, \
             tc.tile_pool(name="wp", bufs=2) as wp, \
             tc.tile_pool(name="psp", bufs=4, space="PSUM") as psp, \
             tc.tile_pool(name="post", bufs=1) as post:
            rhs_sb = const.tile([K_AUG, 256], bf16)
            idx_sb = const.tile([128, NIDX // 16], mybir.dt.int16)
            table = const.tile([128, NE], f32)
            nc.sync.dma_start(out=rhs_sb[:, :], in_=rhs_d[:, :])
            nc.sync.dma_start(out=idx_sb[:, :], in_=idx_d[:, :])
            nc.vector.memset(table[:, 0:1], MASK_D)

            for chunk in range(4):
                w_sb = wp.tile([K_AUG, 8 * 128], bf16, tag="w")
                nc.sync.dma_start(
                    out=w_sb[:, :], in_=w_d[:, chunk * 1024:(chunk + 1) * 1024])
                for i in range(8):
                    jb = chunk * 8 + i
                    pst = psp.tile([128, 256], f32)
                    nc.tensor.matmul(pst[:, :], lhsT=w_sb[:, i * 128:(i + 1) * 128],
                                     rhs=rhs_sb[:, :], start=True, stop=True)
                    dst = table[:, 1 + jb * 256:1 + (jb + 1) * 256]
                    if jb % 2 == 0:
                        nc.vector.tensor_copy(dst, pst[:, :])
                    else:
                        nc.scalar.copy(dst, pst[:, :])

            g = post.tile([128, NIDX], f32)
            nc.gpsimd.ap_gather(out_ap=g[:, :], in_ap=table[:, :],
                                idxs_ap=idx_sb[:, :], channels=128,
                                num_elems=NE, d=1, num_idxs=NIDX)
            phi = post.tile([128, NIDX], f32)
            nc.scalar.activation(phi[:, :], g[:, :],
                                 mybir.ActivationFunctionType.Exp,
                                 scale=-1.0 / 256.0)
            ephi = post.tile([128, NIDX], f32)
            nc.scalar.activation(ephi[:, :], phi[:, :],
                                 mybir.ActivationFunctionType.Exp, scale=-10.0)
            prod = post.tile([128, NIDX], f32)
            nc.vector.tensor_mul(prod[:, :], phi[:, :], ephi[:, :])
            res_sb = post.tile([128, NA, 2], f32)
            prod_v = prod[:, :].rearrange("p (a l) -> p a l", a=NA)
            ephi_v = ephi[:, :].rearrange("p (a l) -> p a l", a=NA)
            nc.vector.tensor_reduce(res_sb[:, :, 0], prod_v,
                                    axis=mybir.AxisListType.X,
                                    op=mybir.AluOpType.add)
            nc.vector.tensor_reduce(res_sb[:, :, 1], ephi_v,
                                    axis=mybir.AxisListType.X,
                                    op=mybir.AluOpType.add)
            nc.sync.dma_start(out=res_d[:, :],
                              in_=res_sb[:, :].rearrange("p a k -> p (a k)"))
    nc.compile()
    return nc


def _hi_lo_bf16(x):
    import ml_dtypes
    hi = x.astype(ml_dtypes.bfloat16)
    lo = (x - hi.astype(np.float32)).astype(ml_dtypes.bfloat16)
    return hi, lo


def _prep_core_inputs(traj, Aset, Y):
    """Host-side shard prep: augmented bf16 operands + bucketed indices."""
    import ml_dtypes
    bf = ml_dtypes.bfloat16
    rhs_all, w_all, idx_all = [], [], []
    for bb in range(B):
        tr = traj[bb, :T_USED, :].astype(np.float32)
        x2_hi, x2_lo = _hi_lo_bf16((tr * tr).sum(-1))
        rhs = np.zeros((K_AUG, T_USED), bf)
        rhs[:64] = tr.T.astype(bf)
        rhs[64] = x2_hi
        rhs[65] = x2_lo
        rhs[66] = bf(1.0)
        rhs[67] = bf(1.0)
        rhs_all.append(rhs)

        t_l = Aset[bb, :, :, 0].astype(np.int64)
        j_l = Aset[bb, :, :, 1].astype(np.int64)
        idx_np = np.zeros((128, NIDX // 16), np.int16)
        for jg in range(8):
            lst = np.zeros(NIDX, np.int16)
            for a in range(NA):
                sel = np.nonzero((j_l[a] & 7) == jg)[0]
                if len(sel) > P_SEG:
                    return None, None, None  # bucket overflow -> fallback
                v = 1 + (j_l[a, sel] >> 3) * 256 + t_l[a, sel]
                lst[a * P_SEG:a * P_SEG + len(sel)] = v.astype(np.int16)
            idx_np[16 * jg:16 * jg + 16, :] = lst.reshape(NIDX // 16, 16).T
        idx_all.append(idx_np)

    for s in range(8):
        ch = s % 2
        Yh = Y[ch * 16:(ch + 1) * 16].astype(np.float32)    # (16, 64, 256)
        y2_hi, y2_lo = _hi_lo_bf16((Yh * Yh).sum(1))        # (16, 256)
        lhsT = np.zeros((K_AUG, NJB, 8, 16), bf)            # k, jb, jg, cc
        lhsT[:64] = (-2.0 * Yh.reshape(16, 64, NJB, 8)
                     .transpose(1, 2, 3, 0)).astype(bf)
        lhsT[64] = bf(1.0)
        lhsT[65] = bf(1.0)
        lhsT[66] = y2_hi.reshape(16, NJB, 8).transpose(1, 2, 0)
        lhsT[67] = y2_lo.reshape(16, NJB, 8).transpose(1, 2, 0)
        w_all.append(np.ascontiguousarray(lhsT.reshape(K_AUG, NJB * 128)))
    return rhs_all, w_all, idx_all


def _kernel_numpy_fallback(traj, Aset, Y, W, b):
    q = Y.shape[2]
    flat = Aset[..., 0].astype(np.int64) * q + Aset[..., 1]
    out = np.zeros((traj.shape[0], W.shape[0]), np.float32)
    for bb in range(traj.shape[0]):
        x2 = (traj[bb] * traj[bb]).sum(-1)
        odds = np.zeros(Y.shape[0], np.float32)
        for c in range(Y.shape[0]):
            y2 = (Y[c] * Y[c]).sum(0)
            xy = traj[bb] @ Y[c]
            D = x2[:, None] + y2[None, :] - 2.0 * xy
            phi = np.exp(-D / 256.0).reshape(-1)[flat[bb]]
            e = np.exp(-phi / 0.1)
            odds[c] = (phi * e / e.sum(-1, keepdims=True)).mean(-1).sum()
        z = odds @ W.T + b
        out[bb] = np.where(z > 0, SELU_L * z, SELU_L * SELU_A * np.expm1(z))
    return out.astype(np.float32)


def kernel(traj, Aset, Y, W, b):
    traj = np.asarray(traj, np.float32)
    Aset = np.asarray(Aset)
    Y = np.asarray(Y, np.float32)
    W = np.asarray(W, np.float32)
    b = np.asarray(b, np.float32)

    if (traj.shape != (B, 4096, D_DIM) or Aset.shape != (B, NA, L, 2)
            or Y.shape != (CTRAJ, D_DIM, Q) or Aset.min() < 0
            or Aset.max() >= 256):
        return _kernel_numpy_fallback(traj, Aset, Y, W, b)

    rhs_all, w_all, idx_all = _prep_core_inputs(traj, Aset, Y)
    if rhs_all is None:
        return _kernel_numpy_fallback(traj, Aset, Y, W, b)

    if "nc" not in _CACHE:
        _CACHE["nc"] = _build_nc()
    nc = _CACHE["nc"]

    from concourse import bass_utils
    in_maps = []
    for s in range(8):
        bb = s // 2
        in_maps.append({"rhs_aug": rhs_all[bb], "lhsT_all": w_all[s],
                        "idxs": idx_all[bb]})
    res = bass_utils.run_bass_kernel_spmd(nc, in_maps, core_ids=list(range(8)))

    num = np.zeros((B, NA, CTRAJ), np.float64)
    den = np.zeros((B, NA, CTRAJ), np.float64)
    for s in range(8):
        bb, ch = s // 2, s % 2
        r = res.results[s]["res"].reshape(8, 16, NA, 2)
        num[bb, :, ch * 16:(ch + 1) * 16] += r[..., 0].sum(0).T
        den[bb, :, ch * 16:(ch + 1) * 16] += r[..., 1].sum(0).T

    odds = (num / den / L).sum(1)                     # (B, CTRAJ)
    z = (odds @ W.T.astype(np.float64) + b).astype(np.float32)
    return np.where(z > 0, SELU_L * z,
                    SELU_L * SELU_A * np.expm1(z)).astype(np.float32)
